# revision 16
# baseline (speedup 1.0000x reference)
"""Trainium2 Bass kernel for LocalWindowAttention.

Model (reference): B=2, S=4096, D=1024, H=16 heads, hd=64, window W=16
(8 left, 7 right), four dim->dim projections (q/k/v/out, torch-Linear
convention y = x @ W.T), per-token windowed softmax attention.

Sharding: 8 cores = 2 batches x 4 sequence chunks of 1024 tokens.  Each
core receives a zero-padded halo of 8 left / 7 right tokens (1039 total)
so K/V at chunk boundaries are computed locally - no collectives.

Host-side preprocessing: x is padded/transposed/cast to xT [D, 1039]
fp16 per core; weights are pre-transposed to W.T [din, dout] fp16.
Padding keys produce k=0 -> score 0 -> exp=1 and v=0, so masking the
sequence edge reduces to subtracting a precomputed count from the
softmax denominator ("adj"), which is exact.  The in-block band mask
(each token attends only keys [i, i+15] of its block's 143-key range)
is applied MULTIPLICATIVELY after exp (0/1 fp16 mask), fused with the
denominator row-sum in one scalar_tensor_tensor per head.

Per-core dataflow (matmuls fp16 operands, fp32 PSUM):
  qT, kT = W.T-stationary matmuls in [dout, t] layout
  v      = xT-stationary matmul in natural [t, dout] layout
  per 128-token block b (2-block software pipeline on PE):
    scores [128 q, 143 keys] per head (pair-packed PSUM banks)
    exp via ScalarE (no accum), band-mask*accum via DVE stt per head,
    normalize via DVE tensor_scalar per head,
    probsT via ONE xbar DMA-transpose [128,16*256] -> [128,32,128],
    AV matmuls (4 head-pairs per PSUM bank), attnT copies on ScalarE,
    out-proj matmuls + DVE copy + fp16 DMA per block.
"""

import numpy as np

import concourse.bass as bass
import concourse.mybir as mybir
import concourse.tile as tile
from concourse import bacc
from concourse.bass_utils import run_bass_kernel_spmd

F16 = mybir.dt.float16
F32 = mybir.dt.float32

B, S, D = 2, 4096, 1024
H, HD = 16, 64
WIN, LP, RP = 16, 8, 7
NCORES = 8
CHUNK = S // 4            # tokens per core
TH = CHUNK + LP + RP      # halo token count (1039)
NB = CHUNK // 128         # q blocks per core (8)
KEYS = 128 + WIN - 1      # keys per block (143)
KPAD = 256                # padded keys per head for xbar transpose
DT = D // 128             # 128-row tiles across D (8)
NVT = (TH + 127) // 128   # v token tiles (9; last has 15 rows)
VTAIL = TH - 128 * (NVT - 1)  # 15

TRACE = False             # test.py may set kernel.TRACE = True
LAST_RESULTS = None       # BassKernelResults of the most recent run

_PROGRAM = None


def _build_program():
    """Build + compile the per-core Bass program (cached)."""
    nc = bacc.Bacc("TRN2", target_bir_lowering=False, debug=False)

    xT_d = nc.dram_tensor("xT", [128, DT, TH], F16, kind="ExternalInput")
    wq_d = nc.dram_tensor("wqT", [D, D], F16, kind="ExternalInput")
    wk_d = nc.dram_tensor("wkT", [D, D], F16, kind="ExternalInput")
    wv_d = nc.dram_tensor("wvT", [D, D], F16, kind="ExternalInput")
    wo_d = nc.dram_tensor("woT", [D, D], F16, kind="ExternalInput")
    adj_d = nc.dram_tensor("adj", [128, NB, H], F32, kind="ExternalInput")
    mask_d = nc.dram_tensor("bandmask", [128, KEYS], F16, kind="ExternalInput")
    out_d = nc.dram_tensor("out", [CHUNK, D], F16, kind="ExternalOutput")

    with tile.TileContext(nc) as tc:
        with (
            tc.tile_pool(name="const", bufs=1) as cpool,
            tc.tile_pool(name="acts", bufs=1) as apool,
            tc.tile_pool(name="wstream", bufs=2 * DT) as wpool,
            tc.tile_pool(name="soft", bufs=2) as spool,
            tc.tile_pool(name="outsb", bufs=3) as opool,
            tc.tile_pool(name="proj_ps", bufs=2, space="PSUM") as proj_ps,
            tc.tile_pool(name="score_ps", bufs=4, space="PSUM") as score_ps,
            tc.tile_pool(name="av_ps", bufs=2, space="PSUM") as av_ps,
        ):
            # ---- constants / inputs resident in SBUF ----
            bandmask = cpool.tile([128, KEYS], F16)
            nc.sync.dma_start(bandmask, mask_d.ap())
            adj_sb = cpool.tile([128, NB, H], F32)
            nc.sync.dma_start(adj_sb, adj_d.ap())

            # xT arrives per k-tile, interleaved with the wq tiles, so the
            # k-streamed first projection can start ~3us in instead of ~12us.
            xT = apool.tile([128, DT, TH], F16)

            qT = apool.tile([128, DT, CHUNK], F16)
            kT = apool.tile([128, DT, TH], F16)
            v_sb = apool.tile([128, NVT, D], F16)
            attnT = apool.tile([128, DT, CHUNK], F16)

            # ping-pong softmax tiles (memset once: xbar transpose reads the
            # full 256-wide rows incl. the never-written [143:256] pad)
            exp_sb = [apool.tile([128, H, KPAD], F16, name=f"exp{i}")
                      for i in range(2)]
            # depth 3: pT[b] is written at iteration b but read by the AV of
            # the 2-block-delayed pipeline stage at iteration b+2.
            pT = [apool.tile([128, 2 * H, 128], F16, name=f"pT{i}")
                  for i in range(3)]
            for t in exp_sb:
                nc.gpsimd.memset(t, 0.0)

            # alternate PSUM->SBUF projection copies between DVE and ScalarE
            copy_state = [0]

            def copy_out(dst, src):
                if copy_state[0] & 1:
                    nc.vector.tensor_copy(dst, src)
                else:
                    nc.scalar.copy(dst, src)
                copy_state[0] += 1

            def load_w(dram):
                tiles = []
                for k in range(DT):
                    wt = wpool.tile([128, D], F16, tag="w", name=f"w_{k}")
                    nc.sync.dma_start(
                        wt, dram.ap().rearrange("(j p) o -> p j o", p=128)[:, k]
                    )
                    tiles.append(wt)
                return tiles

            # ---- qT / kT projections: out [dout_tile, tokens] ----
            def proj_T(w_tiles, dst, tok_off, tok_n):
                chunks = []
                c0 = 0
                while c0 < tok_n:
                    cn = min(512, tok_n - c0)
                    chunks.append((c0, cn))
                    c0 += cn
                for m in range(DT):
                    for (c0, cn) in chunks:
                        ps = proj_ps.tile([128, 512], F32, tag="proj")
                        for k in range(DT):
                            nc.tensor.matmul(
                                ps[:, :cn],
                                w_tiles[k][:, m * 128:(m + 1) * 128],
                                xT[:, k, tok_off + c0: tok_off + c0 + cn],
                                start=(k == 0),
                                stop=(k == DT - 1),
                            )
                        copy_out(dst[:, m, c0:c0 + cn], ps[:, :cn])

            # interleave xT k-tile DMAs with wq tile DMAs
            wq = []
            for k in range(DT):
                nc.sync.dma_start(xT[:, k, :], xT_d.ap()[:, k, :])
                wt = wpool.tile([128, D], F16, tag="w", name=f"wq_{k}")
                nc.sync.dma_start(
                    wt, wq_d.ap().rearrange("(j p) o -> p j o", p=128)[:, k]
                )
                wq.append(wt)

            # ---- qT projection, k-streamed: sweep k outer over 4 m-groups
            # (8 live PSUM tiles borrowed across the three pools) so the PE
            # starts as soon as xT[0]/wq[0] land and streams behind the DMAs.
            for half in (0, 1):
                ps = {}
                for mi in range(4):
                    m = 4 * half + mi
                    for ci in range(2):
                        slot = 2 * mi + ci
                        pool = (proj_ps if slot < 2
                                else score_ps if slot < 6 else av_ps)
                        tg = "proj" if slot < 2 else "sc" if slot < 6 else "av"
                        ps[m, ci] = pool.tile(
                            [128, 512], F32, tag=tg, name=f"qs_{m}_{ci}"
                        )
                for k in range(DT):
                    for mi in range(4):
                        m = 4 * half + mi
                        for ci, c0 in enumerate((0, 512)):
                            nc.tensor.matmul(
                                ps[m, ci][:, :],
                                wq[k][:, m * 128:(m + 1) * 128],
                                xT[:, k, LP + c0: LP + c0 + 512],
                                start=(k == 0),
                                stop=(k == DT - 1),
                            )
                for mi in range(4):
                    m = 4 * half + mi
                    for ci, c0 in enumerate((0, 512)):
                        copy_out(qT[:, m, c0:c0 + 512], ps[m, ci][:, :])

            wk = load_w(wk_d)
            proj_T(wk, kT, 0, TH)

            # ---- v projection: natural [tokens, dout] ----
            wv = load_w(wv_d)
            for j in range(NVT):
                rows = 128 if j < NVT - 1 else VTAIL
                for n in range(2):
                    ps = proj_ps.tile([128, 512], F32, tag="proj")
                    for k in range(DT):
                        nc.tensor.matmul(
                            ps[:rows, :],
                            xT[:, k, j * 128: j * 128 + rows],
                            wv[k][:, n * 512:(n + 1) * 512],
                            start=(k == 0),
                            stop=(k == DT - 1),
                        )
                    copy_out(v_sb[:rows, j, n * 512:(n + 1) * 512], ps[:rows, :])

            wo = load_w(wo_d)

            # Pair same-parity heads: a PSUM bank must only receive matmuls
            # with one PE tile_position (same operand base partition).
            pairs = [(2 * a + l, 2 * a + l + 2)
                     for l in range(2) for a in (0, 2, 4, 6)]

            def emit_score_pair(b, exp, sums, ha, hb):
                """score matmuls -> exp -> mask*accum for one head pair."""
                sc = score_ps.tile([128, 2, KEYS], F32, tag="sc")
                for i, h in enumerate((ha, hb)):
                    l = h & 1
                    nc.tensor.matmul(
                        sc[:, i, :],
                        qT[64 * l:64 * l + 64, h // 2, b * 128:(b + 1) * 128],
                        kT[64 * l:64 * l + 64, h // 2, b * 128: b * 128 + KEYS],
                        start=True,
                        stop=True,
                    )
                # exp of both heads of the pair in one ScalarE op
                nc.scalar.activation(
                    exp[:, ha:hb + 1:2, 0:KEYS],
                    sc,
                    mybir.ActivationFunctionType.Exp,
                    scale=0.125,
                )
                # band mask (multiplicative 0/1) + denominator row-sum
                for h in (ha, hb):
                    nc.vector.scalar_tensor_tensor(
                        exp[:, h, 0:KEYS],
                        exp[:, h, 0:KEYS],
                        1.0,
                        bandmask,
                        mybir.AluOpType.mult,
                        mybir.AluOpType.mult,
                        accum_out=sums[:, h:h + 1],
                    )

            def emit_norm_transpose(b, exp, sums):
                denom = spool.tile([128, H], F32, tag="denom")
                rinv = spool.tile([128, H], F32, tag="rinv")
                nc.vector.tensor_tensor(
                    denom, sums, adj_sb[:, b, :], mybir.AluOpType.subtract
                )
                nc.vector.reciprocal(rinv, denom)
                # normalize: spread the 16 per-head multiplies across DVE,
                # Pool (gpsimd) and ScalarE so no engine exceeds the PE's
                # per-block budget.
                for h in range(H):
                    if h < 8:
                        nc.gpsimd.tensor_scalar_mul(
                            exp[:, h, 0:KEYS], exp[:, h, 0:KEYS],
                            rinv[:, h:h + 1],
                        )
                    elif h < 14:
                        nc.vector.tensor_scalar_mul(
                            exp[:, h, 0:KEYS], exp[:, h, 0:KEYS],
                            rinv[:, h:h + 1],
                        )
                    else:
                        nc.scalar.mul(
                            exp[:, h, 0:KEYS], exp[:, h, 0:KEYS],
                            rinv[:, h:h + 1],
                        )
                # probsT [key%128, 2h+chunk, q] in one xbar DMA transpose
                nc.sync.dma_start_transpose(pT[b % 3], exp)

            def emit_av_group(b, g):
                """AV matmuls for head pairs 4g..4g+3, attnT copy (ScalarE)."""
                pt = pT[b % 3]
                av = av_ps.tile([128, 4, 128], F32, tag="av")
                for j4 in range(4):
                    j = 4 * g + j4
                    for l in range(2):
                        h = 2 * j + l
                        nc.tensor.matmul(
                            av[64 * l:64 * l + 64, j4, :],
                            v_sb[:, b, 64 * h:64 * h + 64],
                            pt[:, 2 * h, :],
                            start=True,
                            stop=False,
                        )
                        nc.tensor.matmul(
                            av[64 * l:64 * l + 64, j4, :],
                            v_sb[0:VTAIL, b + 1, 64 * h:64 * h + 64],
                            pt[0:VTAIL, 2 * h + 1, :],
                            start=False,
                            stop=True,
                        )
                nc.scalar.copy(
                    attnT[:, 4 * g:4 * g + 4, b * 128:(b + 1) * 128], av
                )

            def emit_outproj(b):
                for n in range(2):
                    ps = proj_ps.tile([128, 512], F32, tag="proj")
                    for k in range(DT):
                        nc.tensor.matmul(
                            ps,
                            attnT[:, k, b * 128:(b + 1) * 128],
                            wo[k][:, n * 512:(n + 1) * 512],
                            start=(k == 0),
                            stop=(k == DT - 1),
                        )
                    osb = opool.tile([128, 512], F16, tag="osb")
                    if n == 0:
                        nc.vector.tensor_copy(osb, ps)
                    else:
                        nc.scalar.copy(osb, ps)
                    nc.sync.dma_start(
                        out_d.ap()[b * 128:(b + 1) * 128,
                                   n * 512:(n + 1) * 512],
                        osb,
                    )

            # ---- attention: 2-block software pipeline on the PE stream.
            # AV/out-proj of block b-2 is interleaved BETWEEN the score pairs
            # of block b so the PE has work while ScalarE drains exps (scores
            # reuse the 4 score PSUM banks at a faster rate than exp frees
            # them).
            for b in range(NB):
                prev = b - 2
                exp = exp_sb[b & 1]
                sums = spool.tile([128, H], F32, tag="sums")
                for p in range(4):
                    emit_score_pair(b, exp, sums, *pairs[p])
                if prev >= 0:
                    emit_av_group(prev, 0)
                for p in range(4, 6):
                    emit_score_pair(b, exp, sums, *pairs[p])
                if prev >= 0:
                    emit_av_group(prev, 1)
                for p in range(6, 8):
                    emit_score_pair(b, exp, sums, *pairs[p])
                emit_norm_transpose(b, exp, sums)
                if prev >= 0:
                    emit_outproj(prev)
            for prev in (NB - 2, NB - 1):
                emit_av_group(prev, 0)
                emit_av_group(prev, 1)
                emit_outproj(prev)

    nc.compile()
    return nc


def _get_program():
    global _PROGRAM
    if _PROGRAM is None:
        _PROGRAM = _build_program()
    return _PROGRAM


def _host_inputs(x, Wq, Wk, Wv, Wo):
    """Shard + preprocess full inputs into per-core input maps."""
    x = np.asarray(x, dtype=np.float32)
    wts = {}
    for name, w in (("wqT", Wq), ("wkT", Wk), ("wvT", Wv), ("woT", Wo)):
        wts[name] = np.ascontiguousarray(np.asarray(w, np.float32).T).astype(
            np.float16
        )

    band = np.zeros((128, KEYS), np.float16)
    ii = np.arange(128)[:, None]
    kk = np.arange(KEYS)[None, :]
    band[(kk >= ii) & (kk <= ii + WIN - 1)] = 1.0

    in_maps = []
    for c in range(NCORES):
        bb, chunk = divmod(c, 4)
        g0 = chunk * CHUNK
        lo, hi = g0 - LP, g0 + CHUNK + RP
        xpad = np.zeros((TH, D), np.float32)
        src_lo, src_hi = max(lo, 0), min(hi, S)
        xpad[src_lo - lo: src_hi - lo] = x[bb, src_lo:src_hi]
        # [TH, D] -> [128, DT, TH] (partition p, k-tile j holds feature 128j+p)
        xT = np.ascontiguousarray(
            xpad.T.reshape(DT, 128, TH).transpose(1, 0, 2)
        ).astype(np.float16)

        glob = g0 + np.arange(CHUNK)
        pos = glob[:, None] - LP + np.arange(WIN)[None, :]
        counts = ((pos < 0) | (pos >= S)).sum(axis=1).astype(np.float32)
        adj = np.zeros((128, NB, H), np.float32)
        adj[:, :, :] = counts.reshape(NB, 128).T[:, :, None]

        in_maps.append(
            {"xT": xT, "adj": adj, "bandmask": band, **wts}
        )
    return in_maps


def kernel(x, Wq, Wk, Wv, Wo):
    global LAST_RESULTS
    nc = _get_program()
    in_maps = _host_inputs(x, Wq, Wk, Wv, Wo)
    res = run_bass_kernel_spmd(
        nc, in_maps, core_ids=list(range(NCORES)), trace=TRACE
    )
    LAST_RESULTS = res
    out = np.empty((B, S, D), np.float32)
    for c in range(NCORES):
        bb, chunk = divmod(c, 4)
        out[bb, chunk * CHUNK:(chunk + 1) * CHUNK] = np.asarray(
            res.results[c]["out"], np.float32
        )
    return out


# revision 21
# speedup vs baseline: 1.0362x; 1.0362x over previous
"""Trainium2 Bass kernel for LocalWindowAttention.

Model (reference): B=2, S=4096, D=1024, H=16 heads, hd=64, window W=16
(8 left, 7 right), four dim->dim projections (q/k/v/out, torch-Linear
convention y = x @ W.T), per-token windowed softmax attention.

Sharding: 8 cores = 2 batches x 4 sequence chunks of 1024 tokens.  Each
core receives a zero-padded halo of 8 left / 7 right tokens (1039 total)
so K/V at chunk boundaries are computed locally - no collectives.

Host-side preprocessing: x is padded/transposed/cast to xT [D, 1039]
fp16 per core; weights are pre-transposed to W.T [din, dout] fp16.
Padding keys produce k=0 -> score 0 -> exp=1 and v=0, so masking the
sequence edge reduces to subtracting a precomputed count from the
softmax denominator ("adj"), which is exact.  The in-block band mask
(each token attends only keys [i, i+15] of its block's 143-key range)
is applied MULTIPLICATIVELY after exp (0/1 fp16 mask), fused with the
denominator row-sum in one scalar_tensor_tensor per head.

Per-core dataflow (matmuls fp16 operands, fp32 PSUM):
  qT, kT = W.T-stationary matmuls in [dout, t] layout
  v      = xT-stationary matmul in natural [t, dout] layout
  per 128-token block b (2-block software pipeline on PE):
    scores [128 q, 143 keys] per head (pair-packed PSUM banks)
    exp via ScalarE (no accum), band-mask*accum via DVE stt per head,
    normalize via DVE tensor_scalar per head,
    probsT via ONE xbar DMA-transpose [128,16*256] -> [128,32,128],
    AV matmuls (4 head-pairs per PSUM bank), attnT copies on ScalarE,
    out-proj matmuls + DVE copy + fp16 DMA per block.
"""

import numpy as np

import concourse.bass as bass
import concourse.mybir as mybir
import concourse.tile as tile
from concourse import bacc
from concourse.bass_utils import run_bass_kernel_spmd

F16 = mybir.dt.float16
F32 = mybir.dt.float32

B, S, D = 2, 4096, 1024
H, HD = 16, 64
WIN, LP, RP = 16, 8, 7
NCORES = 8
CHUNK = S // 4            # tokens per core
TH = CHUNK + LP + RP      # halo token count (1039)
NB = CHUNK // 128         # q blocks per core (8)
KEYS = 128 + WIN - 1      # keys per block (143)
KPAD = 256                # padded keys per head for xbar transpose
DT = D // 128             # 128-row tiles across D (8)
NVT = (TH + 127) // 128   # v token tiles (9; last has 15 rows)
VTAIL = TH - 128 * (NVT - 1)  # 15

TRACE = False             # test.py may set kernel.TRACE = True
LAST_RESULTS = None       # BassKernelResults of the most recent run

_PROGRAM = None


def _build_program():
    """Build + compile the per-core Bass program (cached)."""
    nc = bacc.Bacc("TRN2", target_bir_lowering=False, debug=False)

    xT_d = nc.dram_tensor("xT", [128, DT, TH], F16, kind="ExternalInput")
    wq_d = nc.dram_tensor("wqT", [D, D], F16, kind="ExternalInput")
    wk_d = nc.dram_tensor("wkT", [D, D], F16, kind="ExternalInput")
    wv_d = nc.dram_tensor("wvT", [D, D], F16, kind="ExternalInput")
    wo_d = nc.dram_tensor("woT", [D, D], F16, kind="ExternalInput")
    adj_d = nc.dram_tensor("adj", [128, NB, H], F32, kind="ExternalInput")
    mask_d = nc.dram_tensor("bandmask", [128, KEYS], F16, kind="ExternalInput")
    out_d = nc.dram_tensor("out", [CHUNK, D], F16, kind="ExternalOutput")

    with tile.TileContext(nc) as tc:
        with (
            tc.tile_pool(name="const", bufs=1) as cpool,
            tc.tile_pool(name="acts", bufs=1) as apool,
            tc.tile_pool(name="wstream", bufs=2 * DT) as wpool,
            tc.tile_pool(name="soft", bufs=2) as spool,
            tc.tile_pool(name="outsb", bufs=3) as opool,
            tc.tile_pool(name="proj_ps", bufs=2, space="PSUM") as proj_ps,
            tc.tile_pool(name="score_ps", bufs=4, space="PSUM") as score_ps,
            tc.tile_pool(name="av_ps", bufs=2, space="PSUM") as av_ps,
        ):
            # ---- constants / inputs resident in SBUF ----
            bandmask = cpool.tile([128, KEYS], F16)
            nc.sync.dma_start(bandmask, mask_d.ap())
            adj_sb = cpool.tile([128, NB, H], F32)
            nc.sync.dma_start(adj_sb, adj_d.ap())

            # xT arrives per k-tile, interleaved with the wq tiles, so the
            # k-streamed first projection can start ~3us in instead of ~12us.
            xT = apool.tile([128, DT, TH], F16)

            qT = apool.tile([128, DT, CHUNK], F16)
            kT = apool.tile([128, DT, TH], F16)
            v_sb = apool.tile([128, NVT, D], F16)
            attnT = apool.tile([128, DT, CHUNK], F16)

            # ping-pong softmax tiles (memset once: xbar transpose reads the
            # full 256-wide rows incl. the never-written [143:256] pad)
            exp_sb = [apool.tile([128, H, KPAD], F16, name=f"exp{i}")
                      for i in range(2)]
            # depth 4: pT[b] is written at iteration b but read by the AV of
            # the 3-block-delayed pipeline stage at iteration b+3.
            pT = [apool.tile([128, 2 * H, 128], F16, name=f"pT{i}")
                  for i in range(4)]
            for t in exp_sb:
                nc.gpsimd.memset(t, 0.0)

            # alternate PSUM->SBUF projection copies between DVE and ScalarE
            copy_state = [0]

            def copy_out(dst, src):
                if copy_state[0] & 1:
                    nc.vector.tensor_copy(dst, src)
                else:
                    nc.scalar.copy(dst, src)
                copy_state[0] += 1

            def load_w(dram):
                tiles = []
                for k in range(DT):
                    wt = wpool.tile([128, D], F16, tag="w", name=f"w_{k}")
                    nc.sync.dma_start(
                        wt, dram.ap().rearrange("(j p) o -> p j o", p=128)[:, k]
                    )
                    tiles.append(wt)
                return tiles

            # ---- qT / kT projections: out [dout_tile, tokens] ----
            def proj_T(w_tiles, dst, tok_off, tok_n):
                chunks = []
                c0 = 0
                while c0 < tok_n:
                    cn = min(512, tok_n - c0)
                    chunks.append((c0, cn))
                    c0 += cn
                for m in range(DT):
                    for (c0, cn) in chunks:
                        ps = proj_ps.tile([128, 512], F32, tag="proj")
                        for k in range(DT):
                            nc.tensor.matmul(
                                ps[:, :cn],
                                w_tiles[k][:, m * 128:(m + 1) * 128],
                                xT[:, k, tok_off + c0: tok_off + c0 + cn],
                                start=(k == 0),
                                stop=(k == DT - 1),
                            )
                        copy_out(dst[:, m, c0:c0 + cn], ps[:, :cn])

            # interleave xT k-tile DMAs with wq tile DMAs
            wq = []
            for k in range(DT):
                nc.sync.dma_start(xT[:, k, :], xT_d.ap()[:, k, :])
                wt = wpool.tile([128, D], F16, tag="w", name=f"wq_{k}")
                nc.sync.dma_start(
                    wt, wq_d.ap().rearrange("(j p) o -> p j o", p=128)[:, k]
                )
                wq.append(wt)

            # ---- qT projection, k-streamed: sweep k outer over 4 m-groups
            # (8 live PSUM tiles borrowed across the three pools) so the PE
            # starts as soon as xT[0]/wq[0] land and streams behind the DMAs.
            for half in (0, 1):
                ps = {}
                for mi in range(4):
                    m = 4 * half + mi
                    for ci in range(2):
                        slot = 2 * mi + ci
                        pool = (proj_ps if slot < 2
                                else score_ps if slot < 6 else av_ps)
                        tg = "proj" if slot < 2 else "sc" if slot < 6 else "av"
                        ps[m, ci] = pool.tile(
                            [128, 512], F32, tag=tg, name=f"qs_{m}_{ci}"
                        )
                for k in range(DT):
                    for mi in range(4):
                        m = 4 * half + mi
                        for ci, c0 in enumerate((0, 512)):
                            nc.tensor.matmul(
                                ps[m, ci][:, :],
                                wq[k][:, m * 128:(m + 1) * 128],
                                xT[:, k, LP + c0: LP + c0 + 512],
                                start=(k == 0),
                                stop=(k == DT - 1),
                            )
                for mi in range(4):
                    m = 4 * half + mi
                    for ci, c0 in enumerate((0, 512)):
                        copy_out(qT[:, m, c0:c0 + 512], ps[m, ci][:, :])

            wk = load_w(wk_d)
            proj_T(wk, kT, 0, TH)

            # ---- v projection: natural [tokens, dout] ----
            wv = load_w(wv_d)
            for j in range(NVT):
                rows = 128 if j < NVT - 1 else VTAIL
                for n in range(2):
                    ps = proj_ps.tile([128, 512], F32, tag="proj")
                    for k in range(DT):
                        nc.tensor.matmul(
                            ps[:rows, :],
                            xT[:, k, j * 128: j * 128 + rows],
                            wv[k][:, n * 512:(n + 1) * 512],
                            start=(k == 0),
                            stop=(k == DT - 1),
                        )
                    copy_out(v_sb[:rows, j, n * 512:(n + 1) * 512], ps[:rows, :])

            wo = load_w(wo_d)

            # Pair same-parity heads: a PSUM bank must only receive matmuls
            # with one PE tile_position (same operand base partition).
            pairs = [(2 * a + l, 2 * a + l + 2)
                     for l in range(2) for a in (0, 2, 4, 6)]

            def emit_score_pair(b, exp, sums, ha, hb):
                """score matmuls -> exp -> mask*accum for one head pair."""
                sc = score_ps.tile([128, 2, KEYS], F32, tag="sc")
                for i, h in enumerate((ha, hb)):
                    l = h & 1
                    nc.tensor.matmul(
                        sc[:, i, :],
                        qT[64 * l:64 * l + 64, h // 2, b * 128:(b + 1) * 128],
                        kT[64 * l:64 * l + 64, h // 2, b * 128: b * 128 + KEYS],
                        start=True,
                        stop=True,
                    )
                # exp of both heads of the pair in one ScalarE op
                nc.scalar.activation(
                    exp[:, ha:hb + 1:2, 0:KEYS],
                    sc,
                    mybir.ActivationFunctionType.Exp,
                    scale=0.125,
                )
                # band mask (multiplicative 0/1) + denominator row-sum
                for h in (ha, hb):
                    nc.vector.scalar_tensor_tensor(
                        exp[:, h, 0:KEYS],
                        exp[:, h, 0:KEYS],
                        1.0,
                        bandmask,
                        mybir.AluOpType.mult,
                        mybir.AluOpType.mult,
                        accum_out=sums[:, h:h + 1],
                    )

            def emit_norm_transpose(b, exp, sums):
                denom = spool.tile([128, H], F32, tag="denom")
                rinv = spool.tile([128, H], F32, tag="rinv")
                nc.vector.tensor_tensor(
                    denom, sums, adj_sb[:, b, :], mybir.AluOpType.subtract
                )
                nc.vector.reciprocal(rinv, denom)
                # normalize: spread the 16 per-head multiplies across DVE,
                # Pool (gpsimd) and ScalarE so no engine exceeds the PE's
                # per-block budget.
                for h in range(H):
                    if h < 8:
                        nc.gpsimd.tensor_scalar_mul(
                            exp[:, h, 0:KEYS], exp[:, h, 0:KEYS],
                            rinv[:, h:h + 1],
                        )
                    elif h < 14:
                        nc.vector.tensor_scalar_mul(
                            exp[:, h, 0:KEYS], exp[:, h, 0:KEYS],
                            rinv[:, h:h + 1],
                        )
                    else:
                        nc.scalar.mul(
                            exp[:, h, 0:KEYS], exp[:, h, 0:KEYS],
                            rinv[:, h:h + 1],
                        )
                # probsT [key%128, 2h+chunk, q] in one xbar DMA transpose
                nc.sync.dma_start_transpose(pT[b % 4], exp)

            def emit_av_group(b, g):
                """AV matmuls for head pairs 4g..4g+3, attnT copy (ScalarE)."""
                pt = pT[b % 4]
                av = av_ps.tile([128, 4, 128], F32, tag="av")
                for j4 in range(4):
                    j = 4 * g + j4
                    for l in range(2):
                        h = 2 * j + l
                        nc.tensor.matmul(
                            av[64 * l:64 * l + 64, j4, :],
                            v_sb[:, b, 64 * h:64 * h + 64],
                            pt[:, 2 * h, :],
                            start=True,
                            stop=False,
                        )
                        nc.tensor.matmul(
                            av[64 * l:64 * l + 64, j4, :],
                            v_sb[0:VTAIL, b + 1, 64 * h:64 * h + 64],
                            pt[0:VTAIL, 2 * h + 1, :],
                            start=False,
                            stop=True,
                        )
                nc.scalar.copy(
                    attnT[:, 4 * g:4 * g + 4, b * 128:(b + 1) * 128], av
                )

            def emit_outproj(b):
                for n in range(2):
                    ps = proj_ps.tile([128, 512], F32, tag="proj")
                    for k in range(DT):
                        nc.tensor.matmul(
                            ps,
                            attnT[:, k, b * 128:(b + 1) * 128],
                            wo[k][:, n * 512:(n + 1) * 512],
                            start=(k == 0),
                            stop=(k == DT - 1),
                        )
                    osb = opool.tile([128, 512], F16, tag="osb")
                    if n == 0:
                        nc.vector.tensor_copy(osb, ps)
                    else:
                        nc.scalar.copy(osb, ps)
                    nc.sync.dma_start(
                        out_d.ap()[b * 128:(b + 1) * 128,
                                   n * 512:(n + 1) * 512],
                        osb,
                    )

            # ---- attention: 2-block software pipeline on the PE stream.
            # AV/out-proj of block b-2 is interleaved BETWEEN the score pairs
            # of block b so the PE has work while ScalarE drains exps (scores
            # reuse the 4 score PSUM banks at a faster rate than exp frees
            # them).
            for b in range(NB):
                prev = b - 3
                exp = exp_sb[b & 1]
                sums = spool.tile([128, H], F32, tag="sums")
                for p in range(4):
                    emit_score_pair(b, exp, sums, *pairs[p])
                if prev >= 0:
                    emit_av_group(prev, 0)
                for p in range(4, 6):
                    emit_score_pair(b, exp, sums, *pairs[p])
                if prev >= 0:
                    emit_av_group(prev, 1)
                for p in range(6, 8):
                    emit_score_pair(b, exp, sums, *pairs[p])
                emit_norm_transpose(b, exp, sums)
                if prev >= 0:
                    emit_outproj(prev)
            for prev in (NB - 3, NB - 2, NB - 1):
                emit_av_group(prev, 0)
                emit_av_group(prev, 1)
                emit_outproj(prev)

    nc.compile()
    return nc


def _get_program():
    global _PROGRAM
    if _PROGRAM is None:
        _PROGRAM = _build_program()
    return _PROGRAM


def _host_inputs(x, Wq, Wk, Wv, Wo):
    """Shard + preprocess full inputs into per-core input maps."""
    x = np.asarray(x, dtype=np.float32)
    wts = {}
    for name, w in (("wqT", Wq), ("wkT", Wk), ("wvT", Wv), ("woT", Wo)):
        wts[name] = np.ascontiguousarray(np.asarray(w, np.float32).T).astype(
            np.float16
        )

    band = np.zeros((128, KEYS), np.float16)
    ii = np.arange(128)[:, None]
    kk = np.arange(KEYS)[None, :]
    band[(kk >= ii) & (kk <= ii + WIN - 1)] = 1.0

    in_maps = []
    for c in range(NCORES):
        bb, chunk = divmod(c, 4)
        g0 = chunk * CHUNK
        lo, hi = g0 - LP, g0 + CHUNK + RP
        xpad = np.zeros((TH, D), np.float32)
        src_lo, src_hi = max(lo, 0), min(hi, S)
        xpad[src_lo - lo: src_hi - lo] = x[bb, src_lo:src_hi]
        # [TH, D] -> [128, DT, TH] (partition p, k-tile j holds feature 128j+p)
        xT = np.ascontiguousarray(
            xpad.T.reshape(DT, 128, TH).transpose(1, 0, 2)
        ).astype(np.float16)

        glob = g0 + np.arange(CHUNK)
        pos = glob[:, None] - LP + np.arange(WIN)[None, :]
        counts = ((pos < 0) | (pos >= S)).sum(axis=1).astype(np.float32)
        adj = np.zeros((128, NB, H), np.float32)
        adj[:, :, :] = counts.reshape(NB, 128).T[:, :, None]

        in_maps.append(
            {"xT": xT, "adj": adj, "bandmask": band, **wts}
        )
    return in_maps


def kernel(x, Wq, Wk, Wv, Wo):
    global LAST_RESULTS
    nc = _get_program()
    in_maps = _host_inputs(x, Wq, Wk, Wv, Wo)
    res = run_bass_kernel_spmd(
        nc, in_maps, core_ids=list(range(NCORES)), trace=TRACE
    )
    LAST_RESULTS = res
    out = np.empty((B, S, D), np.float32)
    for c in range(NCORES):
        bb, chunk = divmod(c, 4)
        out[bb, chunk * CHUNK:(chunk + 1) * CHUNK] = np.asarray(
            res.results[c]["out"], np.float32
        )
    return out
